# revision 15
# baseline (speedup 1.0000x reference)
"""v3: engine-balanced key-compacted sparse causal attention.

Changes vs v2 (36.4us):
- PV matmuls reoriented: out oT[128q, 65] with p-slices stationary.
  Streams 65 rows per (block, q-sub) instead of 512 per block; Ldweights
  is ~free, so PV tensor-engine time drops ~2.6x.
- exp split between ACT (exact) and DVE (Schraudolph bit-trick:
  p = bitcast_bf16(int16(A*s + BIAS)); DVE converts f32->int16 with
  truncation, BIAS tuned for that). qt is pre-scaled by A*scale and a
  65th contract row carries BIAS, so s' = A*s + BIAS lands in PSUM
  straight from the QK matmul. A DVE group's exp+causal-mask is then a
  single scalar_tensor_tensor per block: p_i16 = (iota >= th) * s',
  truncated to int16 (masked scores -> int16 0 -> bf16 +0.0). ACT
  groups undo the affine via activation(scale=1/A, bias=-BIAS/A) and
  get post-exp STT masks only on boundary blocks.
- greedy engine assignment prefers DVE for mask-carrying groups and
  balances ACT/DVE/evacuation load.
- p/v in bf16 (PV inputs); q/k stay f32r for score accuracy.
- output dram [8, 128, 260] f32, unscrambled on host; iota on-device.
"""

import sys

import numpy as np

try:
    import concourse  # noqa: F401
except ImportError:  # pragma: no cover
    sys.path.insert(0, "/opt/trn_rl_repo")

B, T, D = 4, 4096, 64
DK = D + 1          # contract dim: 64 data rows + 1 bias row
NCORES = 8
QS_N = 8
QSB = 512
KB = 128
GRP = 2
NEG_BIG = 1e9
QS_ORDER = (0, 1, 7, 6, 5, 4, 3, 2)

A_CONST = float(np.float32(2.0 ** 7 / np.log(2.0)))    # 184.665
CORR = 5.1
BIAS_CONST = float(np.float32(127.0 * 2 ** 7 - CORR))  # 16250.9
INV_A = float(np.float32(1.0) / np.float32(A_CONST))
ACT_BIAS = -BIAS_CONST * INV_A

_compiled = {}


def _build_nc(blocks, slotsets):
    import concourse.bass as bass
    import concourse.mybir as mybir
    import concourse.tile as tile
    from concourse import bacc

    f32 = mybir.dt.float32
    f32r = mybir.dt.float32r
    bf16 = mybir.dt.bfloat16
    fp16 = mybir.dt.float16
    i16 = mybir.dt.int16
    nb_tot = blocks[-1]

    # th column 0 = always-keep (-1e30); slots from 1
    th_col = {}
    c = 1
    for qs in range(QS_N):
        for kb in slotsets[qs]:
            th_col[(qs, kb)] = c
            c += 1
    ns_tot = c

    nc = bacc.Bacc(None, target_bir_lowering=False, debug=False)
    qt_d = nc.declare_dram_parameter("qt", [DK, T], f32r, isOutput=False)
    kt_d = nc.declare_dram_parameter("kt", [DK, nb_tot * KB], f32r,
                                     isOutput=False)
    vp_d = nc.declare_dram_parameter("vp", [KB, nb_tot * 65], bf16,
                                     isOutput=False)
    th_d = nc.declare_dram_parameter("th", [KB, ns_tot], f32, isOutput=False)
    o_d = nc.declare_dram_parameter("o", [QS_N, KB, 4 * 65], f32,
                                    isOutput=True)

    # group list in processing order
    items = []
    for qs in QS_ORDER:
        nkb = blocks[qs]
        for g0 in range(0, nkb, GRP):
            items.append((qs, g0 == 0, nkb,
                          list(range(g0, min(g0 + GRP, nkb)))))

    # per-BLOCK ACT/DVE assignment. A DVE block is one fused STT
    # (exp+causal mask in a single op), so mask-carrying blocks go to
    # DVE; clean blocks go to whichever engine has the lower running
    # total. Alternating engines within a group also keeps both engines
    # fed every group. Evacuations balance the same way.
    blk_eng = {}
    evac_eng = {}
    act_t, dve_t = 0.0, 0.0
    CA, CD, CE_A, CE_D = 615.0, 620.0, 402.0, 420.0
    for qs, _, nkb, gkbs in items:
        for kb in gkbs:
            if kb in slotsets[qs] or act_t + CA > dve_t + CD:
                blk_eng[(qs, kb)] = "dve"
                dve_t += CD
            else:
                blk_eng[(qs, kb)] = "act"
                act_t += CA
        if gkbs[-1] == nkb - 1:
            if act_t + CE_A <= dve_t + CE_D:
                evac_eng[qs] = "act"
                act_t += CE_A
            else:
                evac_eng[qs] = "dve"
                dve_t += CE_D

    with tile.TileContext(nc) as tc:
        with (
            tc.tile_pool(name="const", bufs=1) as cpool,
            tc.tile_pool(name="pt", bufs=4) as ppool,
            tc.tile_pool(name="ob", bufs=2) as obpool,
            tc.tile_pool(name="ps", bufs=2, space=bass.MemorySpace.PSUM) as spool,
            tc.tile_pool(name="po", bufs=1, space=bass.MemorySpace.PSUM) as opool,
        ):
            qt = cpool.tile([DK, T], f32r)
            kt = cpool.tile([DK, nb_tot * KB], f32r)
            vp = cpool.tile([KB, nb_tot * 65], bf16)
            th = cpool.tile([KB, ns_tot], f32)
            io = cpool.tile([KB, QSB], fp16)
            actb = cpool.tile([KB, 1], f32)
            nc.gpsimd.memset(actb[:], ACT_BIAS)

            k1 = min(blocks[QS_ORDER[1]], nb_tot)
            nc.sync.dma_start(kt[:, 0:k1 * KB], kt_d[:, 0:k1 * KB])
            nc.sync.dma_start(qt[:, 0:2 * QSB], qt_d[:, 0:2 * QSB])
            nc.sync.dma_start(vp[:, 0:k1 * 65], vp_d[:, 0:k1 * 65])
            nc.sync.dma_start(th[:], th_d[:])
            if k1 < nb_tot:
                nc.sync.dma_start(kt[:, k1 * KB:], kt_d[:, k1 * KB:])
                nc.sync.dma_start(vp[:, k1 * 65:], vp_d[:, k1 * 65:])
            nc.sync.dma_start(qt[:, 7 * QSB:T], qt_d[:, 7 * QSB:T])
            nc.sync.dma_start(qt[:, 2 * QSB:7 * QSB], qt_d[:, 2 * QSB:7 * QSB])
            nc.gpsimd.iota(io[:], pattern=[[1, QSB]], base=0,
                           channel_multiplier=0,
                           allow_small_or_imprecise_dtypes=True)

            o_of = {}
            pending = []
            LAG = 2  # groups of PV emission lag so p is ready when PV
            # instructions reach the PE queue (4-deep wait queue would
            # otherwise head-of-line block the PE sequencer)

            def emit_pv(qs, gkbs, nkb, p):
                # each q-sub chain accumulates in its own 2KB PSUM bank —
                # concurrent accumulation groups must not share a bank.
                oacc = o_of[qs]
                for j, kb in enumerate(gkbs):
                    for sub in range(4):
                        nc.tensor.matmul(
                            oacc[:, sub * 512:sub * 512 + 65],
                            p[:, j * QSB + sub * KB:j * QSB + (sub + 1) * KB],
                            vp[:, kb * 65:(kb + 1) * 65],
                            start=(kb == 0), stop=(kb == nkb - 1),
                        )
                if gkbs[-1] == nkb - 1:
                    ob = obpool.tile([KB, 4 * 65], f32, name=f"ob{qs}",
                                     tag="ob")
                    src = oacc[:].rearrange("p (s r) -> p s r", s=4)[:, :, 0:65]
                    dst = ob[:].rearrange("p (s c) -> p s c", s=4)
                    if evac_eng[qs] == "act":
                        nc.scalar.activation(
                            dst, src, mybir.ActivationFunctionType.Copy)
                    else:
                        nc.vector.tensor_copy(dst, src)
                    nc.sync.dma_start(o_d[qs], ob[:])
                    del o_of[qs]

            for idx, (qs, first, nkb, gkbs) in enumerate(items):
                if first:
                    o_of[qs] = opool.tile([KB, 4 * 512], f32,
                                          name=f"oacc{qs}", tag="oacc")
                q0 = qs * QSB
                w = len(gkbs) * QSB
                s = spool.tile([KB, GRP * QSB], f32)
                for j, kb in enumerate(gkbs):
                    nc.tensor.matmul(
                        s[:, j * QSB:(j + 1) * QSB],
                        kt[:, kb * KB:(kb + 1) * KB],
                        qt[:, q0:q0 + QSB],
                        start=True, stop=True,
                    )
                if len(pending) >= LAG:
                    emit_pv(*pending.pop(0))
                p = ppool.tile([KB, GRP * QSB], bf16)
                for j, kb in enumerate(gkbs):
                    sl = slice(j * QSB, (j + 1) * QSB)
                    if blk_eng[(qs, kb)] == "act":
                        nc.scalar.activation(
                            p[:, sl], s[:, sl],
                            mybir.ActivationFunctionType.Exp,
                            scale=INV_A, bias=actb[:],
                        )
                        if kb in slotsets[qs]:
                            col = th_col[(qs, kb)]
                            nc.vector.scalar_tensor_tensor(
                                p[:, sl], io[:], th[:, col:col + 1],
                                p[:, sl],
                                op0=mybir.AluOpType.is_ge,
                                op1=mybir.AluOpType.mult,
                            )
                    else:
                        col = th_col.get((qs, kb), 0)
                        nc.vector.scalar_tensor_tensor(
                            p[:, sl].bitcast(i16),
                            io[:], th[:, col:col + 1], s[:, sl],
                            op0=mybir.AluOpType.is_ge,
                            op1=mybir.AluOpType.mult,
                        )
                pending.append((qs, gkbs, nkb, p))
            for args in pending:
                emit_pv(*args)

    nc.compile()
    return nc


def _plan(vm):
    """blocks[qs]: uniform (max-over-core) 128-key block counts, plus the
    per-qs sets of blocks that need a causal/validity mask slot."""
    lives = []
    for c in range(NCORES):
        b, par = c // 2, c % 2
        lives.append(np.flatnonzero(vm[b])[par::2])
    blocks = []
    for qs in range(QS_N):
        bmax = 1
        for live in lives:
            cnt = int(np.searchsorted(live, (qs + 1) * QSB))
            bmax = max(bmax, -(-cnt // KB))
        blocks.append(bmax)
    for qs in range(1, QS_N):
        blocks[qs] = max(blocks[qs], blocks[qs - 1])
    slotsets = []
    for qs in range(QS_N):
        ss = set()
        for live in lives:
            for kb in range(blocks[qs]):
                blk = live[kb * KB:(kb + 1) * KB]
                if blk.size and blk.max() > qs * QSB:
                    ss.add(kb)
        slotsets.append(tuple(sorted(ss)))
    return blocks, slotsets, lives


def _get_nc(blocks, slotsets):
    key = (tuple(blocks), tuple(slotsets))
    if key not in _compiled:
        _compiled[key] = _build_nc(blocks, slotsets)
    return _compiled[key]


def _host_inputs(query, value, keys, q_mask, v_mask, scale):
    import ml_dtypes

    scale = np.float32(scale)
    q = np.asarray(query, np.float32)
    v = np.asarray(value, np.float32)
    k = np.asarray(keys, np.float32)
    vm = np.asarray(v_mask).astype(bool)

    blocks, slotsets, lives = _plan(vm)
    nb_tot = blocks[-1]
    npad = nb_tot * KB
    ns_tot = 1 + sum(len(s) for s in slotsets)

    in_maps = []
    for c in range(NCORES):
        b = c // 2
        live = lives[c]
        nl = live.size
        k_orig = np.full(npad, 2 * T, np.float32)
        k_orig[:nl] = live
        kc = np.zeros((npad, D), np.float32)
        kc[:nl] = k[b][live]
        vc = np.zeros((npad, 65), np.float32)
        vc[:nl, :64] = v[b][live]
        vc[:nl, 64] = 1.0
        qt = np.empty((DK, T), np.float32)
        qt[:D] = q[b].T * (scale * np.float32(A_CONST))
        qt[D] = 1.0
        kt = np.empty((DK, npad), np.float32)
        kt[:D] = kc.T
        kt[D] = BIAS_CONST
        vp = np.ascontiguousarray(
            vc.reshape(nb_tot, KB, 65).transpose(1, 0, 2).reshape(KB, -1)
        ).astype(ml_dtypes.bfloat16)
        th = np.zeros((KB, ns_tot), np.float32)
        th[:, 0] = -1e30  # always-keep column for unslotted DVE blocks
        col = 1
        for qs in range(QS_N):
            for kb in slotsets[qs]:
                # keep iff query_idx >= k_orig - qs*512  (k <= absolute q)
                th[:, col] = k_orig[kb * KB:(kb + 1) * KB] - qs * QSB
                col += 1
        in_maps.append({"qt": np.ascontiguousarray(qt),
                        "kt": np.ascontiguousarray(kt), "vp": vp,
                        "th": np.ascontiguousarray(th)})
    return in_maps, blocks, slotsets


def _host_gather(results, query, value, keys, q_mask, v_mask, scale):
    q = np.asarray(query, np.float32)
    v = np.asarray(value, np.float32)
    k = np.asarray(keys, np.float32)
    qm = np.asarray(q_mask).astype(bool)
    vm = np.asarray(v_mask).astype(bool)
    scale = np.float32(scale)

    out = np.empty((B, T, D), np.float32)
    for b in range(B):
        oq = results[2 * b]["o"] + results[2 * b + 1]["o"]  # [8, 128, 260]
        oq = oq.reshape(QS_N, KB, 4, 65).transpose(0, 2, 1, 3)
        oq = np.ascontiguousarray(oq).reshape(T, 65)
        l = oq[:, 64]
        out[b] = oq[:, :64] / np.where(l > 0, l, 1.0)[:, None]
        nz = np.flatnonzero(vm[b])
        first = nz[0] if nz.size else T
        if first > 0:
            rows = np.arange(first)
            s = ((q[b, rows] @ k[b].T) * scale).astype(np.float32)
            s = s - np.float32(NEG_BIG)
            s = s.astype(np.float64)
            s -= s.max(axis=1, keepdims=True)
            p = np.exp(s)
            p /= p.sum(axis=1, keepdims=True)
            out[b, rows] = p @ v[b].astype(np.float64)
    out = np.where(qm[..., None], out, np.float32(0.0))
    return out


def kernel(**inputs):
    from concourse.bass_utils import run_bass_kernel_spmd

    in_maps, blocks, slotsets = _host_inputs(**inputs)
    nc = _get_nc(blocks, slotsets)
    res = run_bass_kernel_spmd(nc, in_maps, list(range(NCORES))).results
    return _host_gather(res, **inputs)
